# revision 21
# baseline (speedup 1.0000x reference)
"""AdaptiveGraphLearning forward on 8 Trainium2 NeuronCores.

Data-parallel over batch B=64: each core processes 8 batches; the (N,N)
adjacency parameter and tiny edge-MLP weights are replicated (the forward
pass needs no collectives).

Per-core dataflow (8 batches as 4 pairs):
  - HWDGE-DMA temporal_features as 2MB quarters (128=(b_lo,n) partitions,
    free=(h-quarter,t)), alternating between the SP and ACT rings so both
    stream the whole kernel (~430 GB/s combined).
  - Sum over t per h-quarter: fold chain f32->bf16 (t:128->64->32->16) +
    reduce -> R (128=(b_lo,n), 128=h). Quarter 0's folds ride GpSimd, the
    rest DVE; TensorE transpose -> node_T bf16 (h on partitions). 1/T is
    folded into the W1 halves host-side.
  - Edge MLP per batch b (PSUM f32, SBUF bf16):
      Pi = Wi.T@node_T, Pj = Wj.T@node_T -> one (128h, 128) PSUM tile
        (col halves), one ScalarE evac -> PB bf16.
      xp[h, i*64+j] = PB[h,i] + PB[h,64+j] via broadcast tensor_tensor:
        i-half 0:32 on DVE, 32:64 on GpSimd (batch-wide, 2 instructions).
      x = relu(xp + b1): two ScalarE activations (i halves).
      h2 = W2.T @ x[chunk] -> PSUM partitions [64*(c%2), ..+64) so two
        chunks share one (128,512) PSUM tile; evacuate relu(h2+b2) on
        ScalarE (even chunk-pairs) / DVE (odd).
      F accum: w3_ps += W3pair_d.T @ h2_sb with one-hot block weights
        routing chunk 2d -> F row 2d, 2d+1 -> 2d+1 (0.25 sym folded in).
      W2 runs 2 chunks behind x, W3 4 behind: no engine FIFO head blocks.
  - F(8,512) -> F(64,64) via SBUF->SBUF DMA (identical linearized element
    order), F^T on TensorE, then
      out = (relu(G + F + F^T) + I) / (rowsum + 1e-8)
    with G = 0.25*(ap+ap^T) host-side; relu/+I/rowsum fused into one DVE
    scalar_tensor_tensor with accum_out.
  - Batch-b epilogue stages interleave into batch b+1's chunk loop so no
    engine FIFO head blocks on the SBUF->SBUF reinterpret DMA.

Harness notes: walrus in this container accepts a single semaphore wait
per instruction, so a BIR-level pass splits Tile's multi-wait
instructions onto EventSemaphore carriers; the Tile kernel-tail drain
gets the same treatment at build time.
"""
import sys

sys.path.insert(0, '/opt/trn_rl_repo')

import numpy as np

B, N, H, T = 64, 64, 128, 128
NCORES = 8
B_LOC = B // NCORES      # 8 batches per core
PAIRS = B_LOC // 2       # 4 batch pairs per core
NCH = N // 8             # 8 i-chunks per batch (8 i x 64 j = 512 wide)

_CACHE = {}


def _install_wait_splitter():
    """walrus's per-instruction sync structs hold a single semaphore wait;
    Tile can emit several on one instruction. Split extras onto preceding
    single-wait Drain instructions at the BIR-JSON level."""
    if _CACHE.get('wait_splitter'):
        return
    import json

    import concourse.bass2jax as bass2jax

    orig = bass2jax.compile_bir_kernel

    def split_waits_in_bir(bir_bytes):
        d = json.loads(bir_bytes)
        n_new = [0]
        for fn in d.get("functions", []):
            for bb in fn.get("blocks", []):
                out = []
                for ins in bb.get("instructions", []):
                    si = ins.get("sync_info") or {}
                    waits = si.get("on_wait") or []
                    if len(waits) > 1:
                        for w in waits[:-1]:
                            n_new[0] += 1
                            out.append({
                                "engine": ins["engine"],
                                "ins": [],
                                "name": f"IWS-{n_new[0]}",
                                "opcode": "EventSemaphore",
                                "outs": [],
                                "sync_info": {"on_update": [], "on_wait": [w]},
                            })
                        si["on_wait"] = [waits[-1]]
                    out.append(ins)
                bb["instructions"] = out
        return json.dumps(d).encode()

    def wrapper(ant_bir_str, *a, **kw):
        return orig(split_waits_in_bir(ant_bir_str), *a, **kw)

    bass2jax.compile_bir_kernel = wrapper
    _CACHE['wait_splitter'] = True


def _split_drain_tile_context(tile_mod, mybir, nc):
    """TileContext whose kernel-tail drain splits its semaphore waits across
    sequential Drain instructions (walrus CTRL insts accept one wait)."""
    from concourse.tile import ScopedClock

    class SplitDrainTileContext(tile_mod.TileContext):
        def _drain_and_barrier(self, tick_clock, wait_clock):
            drain_inst = self.nc.sync.drain()
            wait_clock.add_sem_waits(
                drain_inst.ins, ScopedClock({None: tick_clock.global_clock})
            )
            waits = list(drain_inst.ins.sync_info.on_wait)
            if len(waits) > 1:
                drain_inst.ins.sync_info = mybir.SyncInfo(
                    on_wait=waits[:1],
                    on_update=list(drain_inst.ins.sync_info.on_update),
                )
                for i in range(1, len(waits)):
                    extra = self.nc.sync.drain()
                    extra.ins.sync_info = mybir.SyncInfo(
                        on_wait=waits[i : i + 1], on_update=[]
                    )
            self.nc.all_engine_barrier()
            assert self.sems is not None
            popped = self.nc._tile_sem_poison_stack.pop()
            assert popped is self._sem_poison
            self.nc.clear_and_free_semaphores(list(self.sems.allocated().values()))
            self.nc.all_engine_barrier()

    return SplitDrainTileContext(nc)


def build_nc():
    import concourse.bass as bass
    import concourse.tile as tile
    from concourse import mybir
    from contextlib import ExitStack

    f32 = mybir.dt.float32
    bf16 = mybir.dt.bfloat16
    AF = mybir.ActivationFunctionType
    ALU = mybir.AluOpType
    AX = mybir.AxisListType

    nc = bass.Bass()
    tf = nc.declare_dram_parameter("tf", [B_LOC, N, H, T], f32, isOutput=False)
    Wi = nc.declare_dram_parameter("Wi", [H, H], bf16, isOutput=False)
    Wj = nc.declare_dram_parameter("Wj", [H, H], bf16, isOutput=False)
    b1c = nc.declare_dram_parameter("b1c", [H, 1], f32, isOutput=False)
    W2 = nc.declare_dram_parameter("W2", [H, H // 2], bf16, isOutput=False)
    b2p = nc.declare_dram_parameter("b2p", [H, 1], f32, isOutput=False)
    W3p = nc.declare_dram_parameter("W3p", [H, 8 * (NCH // 2)], bf16,
                                    isOutput=False)
    b3c = nc.declare_dram_parameter("b3c", [8, 1], f32, isOutput=False)
    Smask = nc.declare_dram_parameter("Smask", [H, NCH * 8 * N], bf16,
                                      isOutput=False)
    G = nc.declare_dram_parameter("G", [N, N], f32, isOutput=False)
    I64 = nc.declare_dram_parameter("I64", [N, N], f32, isOutput=False)
    I128 = nc.declare_dram_parameter("I128", [H, H], f32, isOutput=False)
    out_ext = nc.declare_dram_parameter("out", [B_LOC, N, N], f32, isOutput=True)

    NOBIAS = _CACHE.get('cfg_nobias', False)
    HQ = H // 4  # 32: h-quarter
    SMW = NCH * 8 * N  # 4096 mask columns

    with _split_drain_tile_context(tile, mybir, nc) as tc, ExitStack() as ctx:
        consts = ctx.enter_context(tc.tile_pool(name="consts", bufs=1))
        tf_pool = ctx.enter_context(tc.tile_pool(name="tf", bufs=7))
        fold_pool = ctx.enter_context(tc.tile_pool(name="fold", bufs=2))
        red_pool = ctx.enter_context(tc.tile_pool(name="red", bufs=2))
        mt_pool = ctx.enter_context(tc.tile_pool(name="mt", bufs=2))
        x_pool = ctx.enter_context(tc.tile_pool(name="x", bufs=6))
        h2_pool = ctx.enter_context(tc.tile_pool(name="h2", bufs=4))
        ff_pool = ctx.enter_context(tc.tile_pool(name="ff", bufs=2))
        ep_pool = ctx.enter_context(tc.tile_pool(name="ep", bufs=2))
        ps_xp = ctx.enter_context(tc.tile_pool(name="ps_xp", bufs=2, space="PSUM"))
        ps_h2 = ctx.enter_context(tc.tile_pool(name="ps_h2", bufs=2, space="PSUM"))
        ps_w3 = ctx.enter_context(tc.tile_pool(name="ps_w3", bufs=1, space="PSUM"))
        ps_t = ctx.enter_context(tc.tile_pool(name="ps_t", bufs=1, space="PSUM"))
        if True:
            def load_consts():
                wi_sb = consts.tile([H, H], bf16, name="wi_sb")
                nc.scalar.dma_start(wi_sb[:], Wi[:])
                wj_sb = consts.tile([H, H], bf16, name="wj_sb")
                nc.scalar.dma_start(wj_sb[:], Wj[:])
                w2_sb = consts.tile([H, H // 2], bf16, name="w2_sb")
                nc.scalar.dma_start(w2_sb[:], W2[:])
                w3_sb = consts.tile([H, 8 * (NCH // 2)], bf16, name="w3_sb")
                nc.scalar.dma_start(w3_sb[:], W3p[:])
                b1_sb = consts.tile([H, 1], f32, name="b1_sb")
                nc.scalar.dma_start(b1_sb[:], b1c[:])
                b2_sb = consts.tile([H, 1], f32, name="b2_sb")
                nc.scalar.dma_start(b2_sb[:], b2p[:])
                b3_sb = consts.tile([8, 1], f32, name="b3_sb")
                nc.scalar.dma_start(b3_sb[:], b3c[:])
                g_sb = consts.tile([N, N], f32, name="g_sb")
                nc.scalar.dma_start(g_sb[:], G[:])
                i64_sb = consts.tile([N, N], f32, name="i64_sb")
                nc.scalar.dma_start(i64_sb[:], I64[:])
                i128_sb = consts.tile([H, H], f32, name="i128_sb")
                nc.scalar.dma_start(i128_sb[:], I128[:])
                return (wi_sb, wj_sb, w2_sb, w3_sb, b1_sb, b2_sb, b3_sb,
                        g_sb, i64_sb, i128_sb)

            def load_pair(c):
                # 4 quarters of 2MB, alternating rings
                qs = []
                for q in range(4):
                    tft = tf_pool.tile([128, HQ, T], f32, name=f"tf{c}_{q}",
                                       tag="tft")
                    eng = nc.sync if q % 2 == 0 else nc.scalar
                    eng.dma_start(
                        tft[:], tf[2 * c : 2 * c + 2, :, q * HQ : (q + 1) * HQ, :])
                    qs.append(tft[:])
                return qs

            # pair0 quarters first on both rings, then the small consts
            pending = load_pair(0)
            (wi_sb, wj_sb, w2_sb, w3_sb, b1_sb, b2_sb, b3_sb, g_sb, i64_sb,
             i128_sb) = load_consts()
            sm_sb = consts.tile([H, SMW], bf16, name="sm_sb")
            nc.sync.dma_start(sm_sb[:, 0 : SMW // 2], Smask[:, 0 : SMW // 2])
            nc.scalar.dma_start(
                sm_sb[:, SMW // 2 : SMW], Smask[:, SMW // 2 : SMW])

            def emit_folds(c, parts):
                # Sum over T: R[p=(b_lo,n), h] = sum_t tf[2c+b_lo, n, h, t].
                # Quarter chains: q2 rides GpSimd, q0/q1/q3 DVE. The DVE
                # chains are emitted FIRST and q2's final reduce (which
                # waits on the slow GpSimd chain) LAST, so DVE's FIFO never
                # head-blocks on GpSimd.
                r_sb = red_pool.tile([128, H], f32, tag="r", name=f"r{c}")

                def chain(q, th, eng, emit_red=True):
                    f1 = fold_pool.tile([128, HQ, 64], bf16, tag=f"f1_{q % 2}",
                                        name=f"f1_{c}_{q}")
                    eng.tensor_tensor(
                        f1[:], th[:, :, 0:64], th[:, :, 64:128], op=ALU.add)
                    f2 = fold_pool.tile([128, HQ, 32], bf16, tag=f"f2_{q % 2}",
                                        name=f"f2_{c}_{q}")
                    eng.tensor_tensor(
                        f2[:], f1[:, :, 0:32], f1[:, :, 32:64], op=ALU.add)
                    f3 = fold_pool.tile([128, HQ, 16], bf16, tag=f"f3_{q % 2}",
                                        name=f"f3_{c}_{q}")
                    eng.tensor_tensor(
                        f3[:], f2[:, :, 0:16], f2[:, :, 16:32], op=ALU.add)
                    if emit_red:
                        nc.vector.reduce_sum(
                            r_sb[:, q * HQ : (q + 1) * HQ], f3[:], axis=AX.X)
                    return f3

                for q in (0, 1, 3):
                    chain(q, parts[q], nc.vector)
                f3g = chain(2, parts[2], nc.gpsimd, emit_red=False)
                nc.vector.reduce_sum(r_sb[:, 2 * HQ : 3 * HQ], f3g[:],
                                     axis=AX.X)
                return r_sb

            def batch_tails(b, w3_ps):
                """Epilogue closures for batch b, emitted into the next
                pair's interleaved loop."""
                st = {}

                def s_ff():
                    ff_sb = ff_pool.tile([8, 512], f32, tag="ff",
                                         name=f"ff{b}")
                    if NOBIAS:
                        nc.scalar.activation(ff_sb[:], w3_ps[:], AF.Copy)
                    else:
                        nc.scalar.activation(ff_sb[:], w3_ps[:], AF.Identity,
                                             bias=b3_sb[:])
                    st['ff'] = ff_sb

                def s_fdma():
                    f_sb = ep_pool.tile([N, N], f32, tag="f", name=f"fsb{b}")
                    nc.sync.dma_start(f_sb[:], st['ff'][:])
                    st['f'] = f_sb

                def s_ft():
                    ft_ps = ps_t.tile([N, N], f32, tag="misc", name=f"ft{b}")
                    nc.tensor.transpose(ft_ps[:], st['f'][:], i64_sb[:, :64])
                    ft_sb = ep_pool.tile([N, N], f32, tag="fts",
                                         name=f"fts{b}")
                    nc.scalar.activation(ft_sb[:], ft_ps[:], AF.Copy)
                    st['ft'] = ft_sb

                def s_epi():
                    # GpSimd adds (SBUF-only); reductions/recip on DVE
                    f_sb, ft_sb = st['f'], st['ft']
                    t1 = ep_pool.tile([N, N], f32, tag="t1", name=f"t1_{b}")
                    nc.gpsimd.tensor_tensor(t1[:], f_sb[:], ft_sb[:],
                                            op=ALU.add)
                    t2 = ep_pool.tile([N, N], f32, tag="t2", name=f"t2_{b}")
                    nc.gpsimd.tensor_tensor(t2[:], t1[:], g_sb[:], op=ALU.add)
                    spi = ep_pool.tile([N, N], f32, tag="spi", name=f"spi{b}")
                    rs = ep_pool.tile([N, 1], f32, tag="rs", name=f"rs{b}")
                    nc.vector.scalar_tensor_tensor(
                        spi[:], t2[:], 0.0, i64_sb[:], op0=ALU.max,
                        op1=ALU.add, accum_out=rs[:])
                    rb = ep_pool.tile([N, 1], f32, tag="rb", name=f"rb{b}")
                    nc.vector.tensor_scalar(
                        rb[:], rs[:], scalar1=1e-8, scalar2=None, op0=ALU.add)
                    rec = ep_pool.tile([N, 1], f32, tag="rec", name=f"rec{b}")
                    nc.vector.reciprocal(rec[:], rb[:])
                    o_sb = ep_pool.tile([N, N], f32, tag="o", name=f"o{b}")
                    nc.vector.tensor_scalar(
                        o_sb[:], spi[:], scalar1=rec[:], scalar2=None,
                        op0=ALU.mult)
                    st['o'] = o_sb

                def s_out():
                    nc.sync.dma_start(out_ext[b], st['o'][:])

                return [s_ff, s_fdma, s_ft, s_epi, s_out]

            def mlp_batch(b, mt_sb, tails):
                """Batch MLP with staged lags: xp mask-matmuls pair into
                (128,1024) PSUM tiles, x-relu behind them, W2 3 chunks
                behind, W3 6 behind. All PSUM evacuations ride ScalarE so
                DVE stays free for the fold stream."""
                w3_ps = ps_w3.tile([8, 512], f32, tag="w3", name=f"w3_{b}")
                xps = {}
                xs = {}
                h2ps = {}
                h2sb = {}
                ti = iter(tails)
                for c in range(NCH + 6):
                    if c < NCH:
                        d2 = c // 2
                        if c % 2 == 0:
                            xps[d2] = ps_xp.tile([128, 1024], f32, tag="xp",
                                                 name=f"xp{b}_{d2}")
                        nc.tensor.matmul(
                            xps[d2][:, 512 * (c % 2) : 512 * (c % 2) + 512],
                            mt_sb[:], sm_sb[:, 512 * c : 512 * (c + 1)],
                            start=True, stop=True)
                        if c % 2 == 1:
                            xt = x_pool.tile([128, 1024], bf16, tag="x",
                                             name=f"x{b}_{d2}")
                            nc.scalar.activation(
                                xt[:], xps[d2][:], AF.Relu,
                                bias=0.0 if NOBIAS else b1_sb[:])
                            xs[d2] = xt
                    k = c - 3
                    if 0 <= k < NCH:
                        d = k // 2
                        if k % 2 == 0:
                            h2ps[d] = ps_h2.tile([128, 512], f32, tag="h2ps",
                                                 name=f"h2ps{b}_{d}")
                        nc.tensor.matmul(
                            h2ps[d][64 * (k % 2) : 64 * (k % 2) + 64, :],
                            w2_sb[:],
                            xs[k // 2][:, 512 * (k % 2) : 512 * (k % 2) + 512],
                            start=True, stop=True)
                        if k % 2 == 1:
                            hs = h2_pool.tile([128, 512], bf16, tag="h2",
                                              name=f"h2_{b}_{d}")
                            nc.scalar.activation(
                                hs[:], h2ps[d][:], AF.Relu,
                                bias=0.0 if NOBIAS else b2_sb[:])
                            h2sb[d] = hs
                    k2 = c - 6
                    if 0 <= k2 < NCH and k2 % 2 == 1:
                        d = k2 // 2
                        nc.tensor.matmul(
                            w3_ps[:], w3_sb[:, 8 * d : 8 * d + 8],
                            h2sb[d][:],
                            start=(d == 0), stop=(d == NCH // 2 - 1))
                    if c in (1, 2, 5, 6, 7, 8):
                        stage = next(ti, None)
                        if stage is not None:
                            stage()
                for stage in ti:
                    stage()
                return batch_tails(b, w3_ps)

            tails = []
            for c in range(PAIRS):
                parts = pending
                if c + 1 < PAIRS:
                    pending = load_pair(c + 1)
                r_sb = emit_folds(c, parts)
                # node_T[h, (b_lo, n)] via four quarter transposes: each only
                # waits its own h-quarter's reduce, shortening the pair
                # boundary critical path
                rt_ps = ps_t.tile([128, 128], f32, tag="misc", name=f"rt{c}")
                nc.tensor.transpose(rt_ps[:], r_sb[:], i128_sb[:])
                rt_sb = red_pool.tile([128, 128], bf16, tag="rt_sb",
                                      name=f"rtsb{c}")
                nc.scalar.activation(rt_sb[:], rt_ps[:], AF.Copy)
                # PiT/PjT + MT for BOTH batches up front so the mask
                # matmuls of batch 2c+1 never wait on a fresh ACT evac
                mts = []
                for b_lo in range(2):
                    b = 2 * c + b_lo
                    nodeb = rt_sb[:, 64 * b_lo : 64 * b_lo + 64]
                    pi_ps = ps_t.tile([128, H], f32, tag="misc",
                                      name=f"pi{b}")
                    nc.tensor.matmul(pi_ps[0:64, :], nodeb, wi_sb[:],
                                     start=True, stop=True)
                    nc.tensor.matmul(pi_ps[64:128, :], nodeb, wj_sb[:],
                                     start=True, stop=True)
                    mt_sb = mt_pool.tile([128, H], bf16, tag="mt",
                                         name=f"mt{b}")
                    nc.scalar.activation(mt_sb[:], pi_ps[:], AF.Copy)
                    mts.append(mt_sb)
                for b_lo in range(2):
                    tails = mlp_batch(2 * c + b_lo, mts[b_lo], tails)
            for stage in tails:
                stage()
    return nc


def _get_nc():
    key = ('nc', _CACHE.get('cfg_nobias', False))
    if key not in _CACHE:
        _CACHE[key] = build_nc()
    return _CACHE[key]


def kernel(**inputs):
    import ml_dtypes

    from concourse.bass_utils import run_bass_kernel_spmd

    _install_wait_splitter()

    tf = np.asarray(inputs["temporal_features"], dtype=np.float32)
    ap = np.asarray(inputs["adj_param"], dtype=np.float32)
    W1 = np.asarray(inputs["W1"], dtype=np.float32)
    b1 = np.asarray(inputs["b1"], dtype=np.float32)
    W2 = np.asarray(inputs["W2"], dtype=np.float32)
    b2 = np.asarray(inputs["b2"], dtype=np.float32)
    W3 = np.asarray(inputs["W3"], dtype=np.float32)
    b3 = np.asarray(inputs["b3"], dtype=np.float32)

    bf = ml_dtypes.bfloat16
    Wi = np.ascontiguousarray((W1[:H] / T).astype(bf))
    Wj = np.ascontiguousarray((W1[H:] / T).astype(bf))
    b1c = b1.reshape(H, 1)
    b2p = np.concatenate([b2, b2]).reshape(H, 1)
    # W3 pair-block weights: chunk-pair d reads h2 of chunk 2d on PSUM
    # partitions 0:64 and chunk 2d+1 on 64:128; route each to F row 2d /
    # 2d+1 of the (8,512) accumulator (0.25 sym factor folded in).
    ND = NCH // 2
    W3p = np.zeros((H, ND, 8), np.float32)
    for d in range(ND):
        W3p[0 : H // 2, d, 2 * d] = 0.25 * W3[:, 0]
        W3p[H // 2 : H, d, 2 * d + 1] = 0.25 * W3[:, 0]
    W3p = np.ascontiguousarray(W3p.reshape(H, ND * 8).astype(bf))
    b3c = np.full((8, 1), 0.25 * float(b3[0]), np.float32)
    # xp mask: column (c, il, j) has ones at slots 8c+il and 64+j
    Smask = np.zeros((H, NCH, 8, N), np.float32)
    for c in range(NCH):
        for il in range(8):
            Smask[8 * c + il, c, il, :] = 1.0
    for j in range(N):
        Smask[64 + j, :, :, j] = 1.0
    Smask = np.ascontiguousarray(Smask.reshape(H, NCH * 8 * N).astype(bf))
    G = np.ascontiguousarray(0.25 * (ap + ap.T))
    I64np = np.eye(N, dtype=np.float32)
    I128np = np.eye(H, dtype=np.float32)

    shared = {
        "Wi": Wi, "Wj": Wj, "b1c": b1c,
        "W2": np.ascontiguousarray(W2.astype(bf)),
        "b2p": b2p, "W3p": W3p, "b3c": b3c, "Smask": Smask, "G": G,
        "I64": I64np, "I128": I128np,
    }
    in_maps = [
        {"tf": np.ascontiguousarray(tf[i * B_LOC : (i + 1) * B_LOC]), **shared}
        for i in range(NCORES)
    ]

    _CACHE['cfg_nobias'] = bool(
        not b1.any() and not b2.any() and not b3.any())
    nc = _get_nc()
    res = run_bass_kernel_spmd(nc, in_maps, core_ids=list(range(NCORES)),
                               **_CACHE.get('run_kwargs', {}))
    _CACHE['last_result'] = res
    out = np.concatenate([res.results[i]["out"] for i in range(NCORES)], axis=0)
    return np.ascontiguousarray(out.astype(np.float32))


# revision 22
# speedup vs baseline: 1.0088x; 1.0088x over previous
"""AdaptiveGraphLearning forward on 8 Trainium2 NeuronCores.

Data-parallel over batch B=64: each core processes 8 batches; the (N,N)
adjacency parameter and tiny edge-MLP weights are replicated (the forward
pass needs no collectives).

Per-core dataflow (8 batches as 4 pairs):
  - HWDGE-DMA temporal_features as 2MB quarters (128=(b_lo,n) partitions,
    free=(h-quarter,t)), alternating between the SP and ACT rings so both
    stream the whole kernel (~430 GB/s combined).
  - Sum over t per h-quarter: fold chain f32->bf16 (t:128->64->32->16) +
    reduce -> R (128=(b_lo,n), 128=h). Quarter 0's folds ride GpSimd, the
    rest DVE; TensorE transpose -> node_T bf16 (h on partitions). 1/T is
    folded into the W1 halves host-side.
  - Edge MLP per batch b (PSUM f32, SBUF bf16):
      Pi = Wi.T@node_T, Pj = Wj.T@node_T -> one (128h, 128) PSUM tile
        (col halves), one ScalarE evac -> PB bf16.
      xp[h, i*64+j] = PB[h,i] + PB[h,64+j] via broadcast tensor_tensor:
        i-half 0:32 on DVE, 32:64 on GpSimd (batch-wide, 2 instructions).
      x = relu(xp + b1): two ScalarE activations (i halves).
      h2 = W2.T @ x[chunk] -> PSUM partitions [64*(c%2), ..+64) so two
        chunks share one (128,512) PSUM tile; evacuate relu(h2+b2) on
        ScalarE (even chunk-pairs) / DVE (odd).
      F accum: w3_ps += W3pair_d.T @ h2_sb with one-hot block weights
        routing chunk 2d -> F row 2d, 2d+1 -> 2d+1 (0.25 sym folded in).
      W2 runs 2 chunks behind x, W3 4 behind: no engine FIFO head blocks.
  - F(8,512) -> F(64,64) via SBUF->SBUF DMA (identical linearized element
    order), F^T on TensorE, then
      out = (relu(G + F + F^T) + I) / (rowsum + 1e-8)
    with G = 0.25*(ap+ap^T) host-side; relu/+I/rowsum fused into one DVE
    scalar_tensor_tensor with accum_out.
  - Batch-b epilogue stages interleave into batch b+1's chunk loop so no
    engine FIFO head blocks on the SBUF->SBUF reinterpret DMA.

Harness notes: walrus in this container accepts a single semaphore wait
per instruction, so a BIR-level pass splits Tile's multi-wait
instructions onto EventSemaphore carriers; the Tile kernel-tail drain
gets the same treatment at build time.
"""
import sys

sys.path.insert(0, '/opt/trn_rl_repo')

import numpy as np

B, N, H, T = 64, 64, 128, 128
NCORES = 8
B_LOC = B // NCORES      # 8 batches per core
PAIRS = B_LOC // 2       # 4 batch pairs per core
NCH = N // 8             # 8 i-chunks per batch (8 i x 64 j = 512 wide)

_CACHE = {}


def _install_wait_splitter():
    """walrus's per-instruction sync structs hold a single semaphore wait;
    Tile can emit several on one instruction. Split extras onto preceding
    single-wait Drain instructions at the BIR-JSON level."""
    if _CACHE.get('wait_splitter'):
        return
    import json

    import concourse.bass2jax as bass2jax

    orig = bass2jax.compile_bir_kernel

    def split_waits_in_bir(bir_bytes):
        d = json.loads(bir_bytes)
        n_new = [0]
        for fn in d.get("functions", []):
            for bb in fn.get("blocks", []):
                out = []
                for ins in bb.get("instructions", []):
                    si = ins.get("sync_info") or {}
                    waits = si.get("on_wait") or []
                    if len(waits) > 1:
                        for w in waits[:-1]:
                            n_new[0] += 1
                            out.append({
                                "engine": ins["engine"],
                                "ins": [],
                                "name": f"IWS-{n_new[0]}",
                                "opcode": "EventSemaphore",
                                "outs": [],
                                "sync_info": {"on_update": [], "on_wait": [w]},
                            })
                        si["on_wait"] = [waits[-1]]
                    out.append(ins)
                bb["instructions"] = out
        return json.dumps(d).encode()

    def wrapper(ant_bir_str, *a, **kw):
        return orig(split_waits_in_bir(ant_bir_str), *a, **kw)

    bass2jax.compile_bir_kernel = wrapper
    _CACHE['wait_splitter'] = True


def _split_drain_tile_context(tile_mod, mybir, nc):
    """TileContext whose kernel-tail drain splits its semaphore waits across
    sequential Drain instructions (walrus CTRL insts accept one wait)."""
    from concourse.tile import ScopedClock

    class SplitDrainTileContext(tile_mod.TileContext):
        def _drain_and_barrier(self, tick_clock, wait_clock):
            drain_inst = self.nc.sync.drain()
            wait_clock.add_sem_waits(
                drain_inst.ins, ScopedClock({None: tick_clock.global_clock})
            )
            waits = list(drain_inst.ins.sync_info.on_wait)
            if len(waits) > 1:
                drain_inst.ins.sync_info = mybir.SyncInfo(
                    on_wait=waits[:1],
                    on_update=list(drain_inst.ins.sync_info.on_update),
                )
                for i in range(1, len(waits)):
                    extra = self.nc.sync.drain()
                    extra.ins.sync_info = mybir.SyncInfo(
                        on_wait=waits[i : i + 1], on_update=[]
                    )
            self.nc.all_engine_barrier()
            assert self.sems is not None
            popped = self.nc._tile_sem_poison_stack.pop()
            assert popped is self._sem_poison
            self.nc.clear_and_free_semaphores(list(self.sems.allocated().values()))
            self.nc.all_engine_barrier()

    return SplitDrainTileContext(nc)


def build_nc():
    import concourse.bass as bass
    import concourse.tile as tile
    from concourse import mybir
    from contextlib import ExitStack

    f32 = mybir.dt.float32
    bf16 = mybir.dt.bfloat16
    AF = mybir.ActivationFunctionType
    ALU = mybir.AluOpType
    AX = mybir.AxisListType

    nc = bass.Bass()
    tf = nc.declare_dram_parameter("tf", [B_LOC, N, H, T], f32, isOutput=False)
    # all small constants packed into two tensors -> two DMAs, so the
    # early DMA queues stay clear for temporal_features quarters
    CBF_W = H + H + H // 2 + 8 * (NCH // 2)  # wi|wj|w2|w3p = 352
    CF_W = 1 + 1 + H + N + N + 1             # b1|b2p|i128|g|i64|b3 = 323
    Cbf = nc.declare_dram_parameter("Cbf", [H, CBF_W], bf16, isOutput=False)
    Cf32 = nc.declare_dram_parameter("Cf32", [H, CF_W], f32, isOutput=False)
    Smask = nc.declare_dram_parameter("Smask", [H, NCH * 8 * N], bf16,
                                      isOutput=False)
    out_ext = nc.declare_dram_parameter("out", [B_LOC, N, N], f32, isOutput=True)

    NOBIAS = _CACHE.get('cfg_nobias', False)
    HQ = H // 4  # 32: h-quarter
    SMW = NCH * 8 * N  # 4096 mask columns

    with _split_drain_tile_context(tile, mybir, nc) as tc, ExitStack() as ctx:
        consts = ctx.enter_context(tc.tile_pool(name="consts", bufs=1))
        tf_pool = ctx.enter_context(tc.tile_pool(name="tf", bufs=7))
        fold_pool = ctx.enter_context(tc.tile_pool(name="fold", bufs=2))
        red_pool = ctx.enter_context(tc.tile_pool(name="red", bufs=2))
        mt_pool = ctx.enter_context(tc.tile_pool(name="mt", bufs=2))
        x_pool = ctx.enter_context(tc.tile_pool(name="x", bufs=6))
        h2_pool = ctx.enter_context(tc.tile_pool(name="h2", bufs=4))
        ff_pool = ctx.enter_context(tc.tile_pool(name="ff", bufs=2))
        ep_pool = ctx.enter_context(tc.tile_pool(name="ep", bufs=2))
        ps_xp = ctx.enter_context(tc.tile_pool(name="ps_xp", bufs=2, space="PSUM"))
        ps_h2 = ctx.enter_context(tc.tile_pool(name="ps_h2", bufs=2, space="PSUM"))
        ps_w3 = ctx.enter_context(tc.tile_pool(name="ps_w3", bufs=1, space="PSUM"))
        ps_t = ctx.enter_context(tc.tile_pool(name="ps_t", bufs=1, space="PSUM"))
        if True:
            def load_consts():
                cbf = consts.tile([H, CBF_W], bf16, name="cbf")
                nc.sync.dma_start(cbf[:], Cbf[:])
                cf = consts.tile([H, CF_W], f32, name="cf")
                nc.scalar.dma_start(cf[:], Cf32[:])
                wi_sb = cbf[:, 0:H]
                wj_sb = cbf[:, H : 2 * H]
                w2_sb = cbf[:, 2 * H : 2 * H + H // 2]
                w3_sb = cbf[:, 2 * H + H // 2 : CBF_W]
                b1_sb = cf[:, 0:1]
                b2_sb = cf[:, 1:2]
                i128_sb = cf[:, 2 : 2 + H]
                g_sb = cf[0:N, 2 + H : 2 + H + N]
                i64_sb = cf[0:N, 2 + H + N : 2 + H + 2 * N]
                b3_sb = cf[0:8, CF_W - 1 : CF_W]
                return (wi_sb, wj_sb, w2_sb, w3_sb, b1_sb, b2_sb, b3_sb,
                        g_sb, i64_sb, i128_sb)

            def load_pair(c):
                # 4 quarters of 2MB, alternating rings
                qs = []
                for q in range(4):
                    tft = tf_pool.tile([128, HQ, T], f32, name=f"tf{c}_{q}",
                                       tag="tft")
                    eng = nc.sync if q % 2 == 0 else nc.scalar
                    eng.dma_start(
                        tft[:], tf[2 * c : 2 * c + 2, :, q * HQ : (q + 1) * HQ, :])
                    qs.append(tft[:])
                return qs

            # pair0 quarters first on both rings, then smask halves and
            # the two packed const transfers
            pending = load_pair(0)
            sm_sb = consts.tile([H, SMW], bf16, name="sm_sb")
            nc.sync.dma_start(sm_sb[:, 0 : SMW // 2], Smask[:, 0 : SMW // 2])
            nc.scalar.dma_start(
                sm_sb[:, SMW // 2 : SMW], Smask[:, SMW // 2 : SMW])
            (wi_sb, wj_sb, w2_sb, w3_sb, b1_sb, b2_sb, b3_sb, g_sb, i64_sb,
             i128_sb) = load_consts()

            def emit_folds(c, parts):
                # Sum over T: R[p=(b_lo,n), h] = sum_t tf[2c+b_lo, n, h, t].
                # Quarter chains: q2 rides GpSimd, q0/q1/q3 DVE. The DVE
                # chains are emitted FIRST and q2's final reduce (which
                # waits on the slow GpSimd chain) LAST, so DVE's FIFO never
                # head-blocks on GpSimd.
                r_sb = red_pool.tile([128, H], f32, tag="r", name=f"r{c}")

                def chain(q, th, eng, emit_red=True):
                    f1 = fold_pool.tile([128, HQ, 64], bf16, tag=f"f1_{q % 2}",
                                        name=f"f1_{c}_{q}")
                    eng.tensor_tensor(
                        f1[:], th[:, :, 0:64], th[:, :, 64:128], op=ALU.add)
                    f2 = fold_pool.tile([128, HQ, 32], bf16, tag=f"f2_{q % 2}",
                                        name=f"f2_{c}_{q}")
                    eng.tensor_tensor(
                        f2[:], f1[:, :, 0:32], f1[:, :, 32:64], op=ALU.add)
                    f3 = fold_pool.tile([128, HQ, 16], bf16, tag=f"f3_{q % 2}",
                                        name=f"f3_{c}_{q}")
                    eng.tensor_tensor(
                        f3[:], f2[:, :, 0:16], f2[:, :, 16:32], op=ALU.add)
                    if emit_red:
                        nc.vector.reduce_sum(
                            r_sb[:, q * HQ : (q + 1) * HQ], f3[:], axis=AX.X)
                    return f3

                for q in (0, 1, 3):
                    chain(q, parts[q], nc.vector)
                f3g = chain(2, parts[2], nc.gpsimd, emit_red=False)
                nc.vector.reduce_sum(r_sb[:, 2 * HQ : 3 * HQ], f3g[:],
                                     axis=AX.X)
                return r_sb

            def batch_tails(b, w3_ps):
                """Epilogue closures for batch b, emitted into the next
                pair's interleaved loop."""
                st = {}

                def s_ff():
                    ff_sb = ff_pool.tile([8, 512], f32, tag="ff",
                                         name=f"ff{b}")
                    if NOBIAS:
                        nc.scalar.activation(ff_sb[:], w3_ps[:], AF.Copy)
                    else:
                        nc.scalar.activation(ff_sb[:], w3_ps[:], AF.Identity,
                                             bias=b3_sb[:])
                    st['ff'] = ff_sb

                def s_fdma():
                    f_sb = ep_pool.tile([N, N], f32, tag="f", name=f"fsb{b}")
                    nc.sync.dma_start(f_sb[:], st['ff'][:])
                    st['f'] = f_sb

                def s_ft():
                    ft_ps = ps_t.tile([N, N], f32, tag="misc", name=f"ft{b}")
                    nc.tensor.transpose(ft_ps[:], st['f'][:], i64_sb[:, :64])
                    ft_sb = ep_pool.tile([N, N], f32, tag="fts",
                                         name=f"fts{b}")
                    nc.scalar.activation(ft_sb[:], ft_ps[:], AF.Copy)
                    st['ft'] = ft_sb

                def s_epi():
                    # GpSimd adds (SBUF-only); reductions/recip on DVE
                    f_sb, ft_sb = st['f'], st['ft']
                    t1 = ep_pool.tile([N, N], f32, tag="t1", name=f"t1_{b}")
                    nc.gpsimd.tensor_tensor(t1[:], f_sb[:], ft_sb[:],
                                            op=ALU.add)
                    t2 = ep_pool.tile([N, N], f32, tag="t2", name=f"t2_{b}")
                    nc.gpsimd.tensor_tensor(t2[:], t1[:], g_sb[:], op=ALU.add)
                    spi = ep_pool.tile([N, N], f32, tag="spi", name=f"spi{b}")
                    rs = ep_pool.tile([N, 1], f32, tag="rs", name=f"rs{b}")
                    nc.vector.scalar_tensor_tensor(
                        spi[:], t2[:], 0.0, i64_sb[:], op0=ALU.max,
                        op1=ALU.add, accum_out=rs[:])
                    rb = ep_pool.tile([N, 1], f32, tag="rb", name=f"rb{b}")
                    nc.vector.tensor_scalar(
                        rb[:], rs[:], scalar1=1e-8, scalar2=None, op0=ALU.add)
                    rec = ep_pool.tile([N, 1], f32, tag="rec", name=f"rec{b}")
                    nc.vector.reciprocal(rec[:], rb[:])
                    o_sb = ep_pool.tile([N, N], f32, tag="o", name=f"o{b}")
                    nc.vector.tensor_scalar(
                        o_sb[:], spi[:], scalar1=rec[:], scalar2=None,
                        op0=ALU.mult)
                    st['o'] = o_sb

                def s_out():
                    nc.sync.dma_start(out_ext[b], st['o'][:])

                return [s_ff, s_fdma, s_ft, s_epi, s_out]

            def mlp_batch(b, mt_sb, tails):
                """Batch MLP with staged lags: xp mask-matmuls pair into
                (128,1024) PSUM tiles, x-relu behind them, W2 3 chunks
                behind, W3 6 behind. All PSUM evacuations ride ScalarE so
                DVE stays free for the fold stream."""
                w3_ps = ps_w3.tile([8, 512], f32, tag="w3", name=f"w3_{b}")
                xps = {}
                xs = {}
                h2ps = {}
                h2sb = {}
                ti = iter(tails)
                for c in range(NCH + 6):
                    if c < NCH:
                        d2 = c // 2
                        if c % 2 == 0:
                            xps[d2] = ps_xp.tile([128, 1024], f32, tag="xp",
                                                 name=f"xp{b}_{d2}")
                        nc.tensor.matmul(
                            xps[d2][:, 512 * (c % 2) : 512 * (c % 2) + 512],
                            mt_sb[:], sm_sb[:, 512 * c : 512 * (c + 1)],
                            start=True, stop=True)
                        if c % 2 == 1:
                            xt = x_pool.tile([128, 1024], bf16, tag="x",
                                             name=f"x{b}_{d2}")
                            nc.scalar.activation(
                                xt[:], xps[d2][:], AF.Relu,
                                bias=0.0 if NOBIAS else b1_sb[:])
                            xs[d2] = xt
                    k = c - 3
                    if 0 <= k < NCH:
                        d = k // 2
                        if k % 2 == 0:
                            h2ps[d] = ps_h2.tile([128, 512], f32, tag="h2ps",
                                                 name=f"h2ps{b}_{d}")
                        nc.tensor.matmul(
                            h2ps[d][64 * (k % 2) : 64 * (k % 2) + 64, :],
                            w2_sb[:],
                            xs[k // 2][:, 512 * (k % 2) : 512 * (k % 2) + 512],
                            start=True, stop=True)
                        if k % 2 == 1:
                            hs = h2_pool.tile([128, 512], bf16, tag="h2",
                                              name=f"h2_{b}_{d}")
                            nc.scalar.activation(
                                hs[:], h2ps[d][:], AF.Relu,
                                bias=0.0 if NOBIAS else b2_sb[:])
                            h2sb[d] = hs
                    k2 = c - 6
                    if 0 <= k2 < NCH and k2 % 2 == 1:
                        d = k2 // 2
                        nc.tensor.matmul(
                            w3_ps[:], w3_sb[:, 8 * d : 8 * d + 8],
                            h2sb[d][:],
                            start=(d == 0), stop=(d == NCH // 2 - 1))
                    if c in (1, 2, 5, 6, 7, 8):
                        stage = next(ti, None)
                        if stage is not None:
                            stage()
                for stage in ti:
                    stage()
                return batch_tails(b, w3_ps)

            tails = []
            for c in range(PAIRS):
                parts = pending
                if c + 1 < PAIRS:
                    pending = load_pair(c + 1)
                r_sb = emit_folds(c, parts)
                # node_T[h, (b_lo, n)] via four quarter transposes: each only
                # waits its own h-quarter's reduce, shortening the pair
                # boundary critical path
                rt_ps = ps_t.tile([128, 128], f32, tag="misc", name=f"rt{c}")
                nc.tensor.transpose(rt_ps[:], r_sb[:], i128_sb[:])
                rt_sb = red_pool.tile([128, 128], bf16, tag="rt_sb",
                                      name=f"rtsb{c}")
                nc.scalar.activation(rt_sb[:], rt_ps[:], AF.Copy)
                # PiT/PjT + MT for BOTH batches up front so the mask
                # matmuls of batch 2c+1 never wait on a fresh ACT evac
                mts = []
                for b_lo in range(2):
                    b = 2 * c + b_lo
                    nodeb = rt_sb[:, 64 * b_lo : 64 * b_lo + 64]
                    pi_ps = ps_t.tile([128, H], f32, tag="misc",
                                      name=f"pi{b}")
                    nc.tensor.matmul(pi_ps[0:64, :], nodeb, wi_sb[:],
                                     start=True, stop=True)
                    nc.tensor.matmul(pi_ps[64:128, :], nodeb, wj_sb[:],
                                     start=True, stop=True)
                    mt_sb = mt_pool.tile([128, H], bf16, tag="mt",
                                         name=f"mt{b}")
                    nc.scalar.activation(mt_sb[:], pi_ps[:], AF.Copy)
                    mts.append(mt_sb)
                for b_lo in range(2):
                    tails = mlp_batch(2 * c + b_lo, mts[b_lo], tails)
            for stage in tails:
                stage()
    return nc


def _get_nc():
    key = ('nc', _CACHE.get('cfg_nobias', False))
    if key not in _CACHE:
        _CACHE[key] = build_nc()
    return _CACHE[key]


def kernel(**inputs):
    import ml_dtypes

    from concourse.bass_utils import run_bass_kernel_spmd

    _install_wait_splitter()

    tf = np.asarray(inputs["temporal_features"], dtype=np.float32)
    ap = np.asarray(inputs["adj_param"], dtype=np.float32)
    W1 = np.asarray(inputs["W1"], dtype=np.float32)
    b1 = np.asarray(inputs["b1"], dtype=np.float32)
    W2 = np.asarray(inputs["W2"], dtype=np.float32)
    b2 = np.asarray(inputs["b2"], dtype=np.float32)
    W3 = np.asarray(inputs["W3"], dtype=np.float32)
    b3 = np.asarray(inputs["b3"], dtype=np.float32)

    bf = ml_dtypes.bfloat16
    Wi = (W1[:H] / T).astype(bf)
    Wj = (W1[H:] / T).astype(bf)
    b1c = b1.reshape(H, 1)
    b2p = np.concatenate([b2, b2]).reshape(H, 1)
    # W3 pair-block weights: chunk-pair d reads h2 of chunk 2d on PSUM
    # partitions 0:64 and chunk 2d+1 on 64:128; route each to F row 2d /
    # 2d+1 of the (8,512) accumulator (0.25 sym factor folded in).
    ND = NCH // 2
    W3p = np.zeros((H, ND, 8), np.float32)
    for d in range(ND):
        W3p[0 : H // 2, d, 2 * d] = 0.25 * W3[:, 0]
        W3p[H // 2 : H, d, 2 * d + 1] = 0.25 * W3[:, 0]
    W3p = W3p.reshape(H, ND * 8).astype(bf)
    b3c = np.zeros((H, 1), np.float32)
    b3c[:8] = 0.25 * float(b3[0])
    # xp mask: column (c, il, j) has ones at slots 8c+il and 64+j
    Smask = np.zeros((H, NCH, 8, N), np.float32)
    for c in range(NCH):
        for il in range(8):
            Smask[8 * c + il, c, il, :] = 1.0
    for j in range(N):
        Smask[64 + j, :, :, j] = 1.0
    Smask = np.ascontiguousarray(Smask.reshape(H, NCH * 8 * N).astype(bf))
    G = 0.25 * (ap + ap.T)
    I64np = np.eye(N, dtype=np.float32)
    I128np = np.eye(H, dtype=np.float32)

    Cbf = np.concatenate(
        [Wi, Wj, W2.astype(bf), W3p], axis=1)
    Gp = np.zeros((H, N), np.float32); Gp[:N] = G
    I64p = np.zeros((H, N), np.float32); I64p[:N] = I64np
    Cf32 = np.concatenate(
        [b1c, b2p, I128np, Gp, I64p, b3c], axis=1).astype(np.float32)

    shared = {
        "Cbf": np.ascontiguousarray(Cbf),
        "Cf32": np.ascontiguousarray(Cf32),
        "Smask": Smask,
    }
    in_maps = [
        {"tf": np.ascontiguousarray(tf[i * B_LOC : (i + 1) * B_LOC]), **shared}
        for i in range(NCORES)
    ]

    _CACHE['cfg_nobias'] = bool(
        not b1.any() and not b2.any() and not b3.any())
    nc = _get_nc()
    res = run_bass_kernel_spmd(nc, in_maps, core_ids=list(range(NCORES)),
                               **_CACHE.get('run_kwargs', {}))
    _CACHE['last_result'] = res
    out = np.concatenate([res.results[i]["out"] for i in range(NCORES)], axis=0)
    return np.ascontiguousarray(out.astype(np.float32))
